# revision 13
# baseline (speedup 1.0000x reference)
"""Trainium2 Bass kernel for ragged GQA attention decode (B=16, QL=4, KV=4096,
H=32, KVH=8, D=128, DIM=4096), tensor-parallel over 8 NeuronCores.

Sharding: core c owns q-heads [4c, 4c+4) and kv-head c. wq/wk/wv are
column-split, wo row-split, KV cache split along the kv-head dim. Each core
computes a partial [64, 4096] output (its heads through its wo rows); the
host sums the 8 partials.

The Bass graph is specialized to the actual cache_len values (known on host
at build time), so only the live prefix of the KV cache is ever read.
"""

import math
import sys
import types

import numpy as np

B, QL, KV, H, KVH, D, DIM = 16, 4, 4096, 32, 8, 128, 4096
N_CORES = 8
HQ = H // N_CORES  # q heads per core = 4
COLS = B * HQ * QL  # 256 = (b, h, i) columns of the per-core attention state
THETA = 10000.0
SCALE = 1.0 / math.sqrt(D)
F32 = None  # set after concourse import


def _install_ntff_hook():
    """Make run_bass_kernel_spmd(trace=True) work in this image: register the
    NTFF profile hook that trn_boot could not (antenv.axon_hooks missing)."""
    try:
        from antenv.axon_hooks import get_axon_ntff_profile_hook  # noqa: F401

        return
    except ImportError:
        pass
    try:
        import antenv
        from trn_agent_boot.trn_boot import _ntff_profile_via_ctypes

        hook = _ntff_profile_via_ctypes("/opt/axon/libaxon_pjrt.so")
        mod = types.ModuleType("antenv.axon_hooks")
        mod.get_axon_ntff_profile_hook = lambda: hook
        mod.set_axon_ntff_profile_hook = lambda h: None
        sys.modules["antenv.axon_hooks"] = mod
        antenv.axon_hooks = mod
    except Exception:
        pass


def _sub_ap(ap, free_dims, extra_offset=0):
    """AP with the same tensor/partition dim but custom free [step, count] dims."""
    import concourse.bass as bass

    return bass.AP(
        tensor=ap.tensor, offset=ap.offset + extra_offset, ap=[ap.ap[0]] + free_dims
    )


def _build(cache_len):
    """Build the per-core Bacc graph, specialized to cache_len (np.int array [B])."""
    import concourse.bacc as bacc
    import concourse.mybir as mybir
    import concourse.tile as tile
    from concourse.masks import make_identity
    from contextlib import ExitStack

    f32 = mybir.dt.float32
    Exp = mybir.ActivationFunctionType.Exp

    nc = bacc.Bacc("TRN2", target_bir_lowering=False, debug=False, num_devices=N_CORES)

    x_d = nc.dram_tensor("x", [B * QL, DIM], f32, kind="ExternalInput").ap()
    wq_d = nc.dram_tensor("wq", [DIM, HQ * D], f32, kind="ExternalInput").ap()
    wk_d = nc.dram_tensor("wk", [DIM, D], f32, kind="ExternalInput").ap()
    wv_d = nc.dram_tensor("wv", [DIM, D], f32, kind="ExternalInput").ap()
    wo_d = nc.dram_tensor("wo", [HQ * D, DIM], f32, kind="ExternalInput").ap()
    kT_d = nc.dram_tensor("kT", [B, D, KV], f32, kind="ExternalInput").ap()
    v_d = nc.dram_tensor("v", [B, KV, D], f32, kind="ExternalInput").ap()
    cos_d = nc.dram_tensor("cosb", [B * QL, D // 2], f32, kind="ExternalInput").ap()
    sin_d = nc.dram_tensor("sinb", [B * QL, D // 2], f32, kind="ExternalInput").ap()
    nmask_d = nc.dram_tensor("nmask", [QL, COLS], f32, kind="ExternalInput").ap()
    out_d = nc.dram_tensor("out", [B * QL, DIM], f32, kind="ExternalOutput").ap()

    L0s = [int(v) for v in cache_len]
    nJs = [(L + 127) // 128 for L in L0s]
    max_nJ = max(nJs) if nJs else 0

    with tile.TileContext(nc) as tc, ExitStack() as ctx:
        const = ctx.enter_context(tc.tile_pool(name="const", bufs=1))
        wstream = ctx.enter_context(tc.tile_pool(name="wstream", bufs=2))
        ropep = ctx.enter_context(tc.tile_pool(name="ropep", bufs=1))
        kvp = ctx.enter_context(tc.tile_pool(name="kvp", bufs=2))
        probsp = ctx.enter_context(tc.tile_pool(name="probsp", bufs=2))
        fin = ctx.enter_context(tc.tile_pool(name="fin", bufs=1))
        yp = ctx.enter_context(tc.tile_pool(name="yp", bufs=2))
        # PSUM pools are phase-scoped (stack allocator, 8 banks total)
        psA = ctx.enter_context(ExitStack())
        ptr = psA.enter_context(tc.tile_pool(name="ptr", bufs=2, space="PSUM"))
        pproj = psA.enter_context(tc.tile_pool(name="pproj", bufs=1, space="PSUM"))

        # ---- constants ----
        ident = const.tile([64, 64], f32)
        make_identity(nc, ident)
        ones128 = const.tile([128, 1], f32)
        nc.vector.memset(ones128, 1.0)
        ones4 = const.tile([4, 1], f32)
        nc.vector.memset(ones4, 1.0)
        ones_row = const.tile([1, 128], f32)
        nc.vector.memset(ones_row, 1.0)
        cos_sb = const.tile([64, 64], f32)
        nc.sync.dma_start(out=cos_sb, in_=cos_d)
        sin_sb = const.tile([64, 64], f32)
        nc.sync.dma_start(out=sin_sb, in_=sin_d)
        nmask_sb = const.tile([QL, COLS], f32)
        nc.sync.dma_start(out=nmask_sb, in_=nmask_d)
        x_sb = const.tile([64, DIM], f32)
        nc.sync.dma_start(out=x_sb, in_=x_d)
        # prewarm the ACT exp table
        warm = const.tile([1, 1], f32)
        nc.scalar.activation(out=warm, in_=ones128[0:1, 0:1], func=Exp)

        # ---- x^T: 32 PE transposes of [64,128] -> xT [128, 32, 64] ----
        xT = const.tile([128, 32, 64], f32)
        for g in range(4):
            pt = ptr.tile([128, 8, 64], f32, tag="ptx")
            for j in range(8):
                k = g * 8 + j
                nc.tensor.transpose(pt[:, j], x_sb[:, k * 128 : (k + 1) * 128], ident)
            nc.vector.tensor_copy(out=xT[:, g * 8 : (g + 1) * 8], in_=pt)

        # ---- QKV projections (orientation: out natural [64, cols]) ----
        wk_sb = const.tile([128, 32, D], f32)
        nc.sync.dma_start(out=wk_sb, in_=wk_d.rearrange("(n p) d -> p n d", p=128))
        wv_sb = const.tile([128, 32, D], f32)
        nc.sync.dma_start(out=wv_sb, in_=wv_d.rearrange("(n p) d -> p n d", p=128))

        xq_ps = pproj.tile([64, HQ * D], f32)
        xk_ps = pproj.tile([64, D], f32)
        xv_ps = pproj.tile([64, D], f32)
        for g in range(4):
            wq_t = wstream.tile([128, 8, HQ * D], f32, tag="w")
            nc.sync.dma_start(
                out=wq_t,
                in_=wq_d[g * 1024 : (g + 1) * 1024, :].rearrange(
                    "(n p) d -> p n d", p=128
                ),
            )
            for j in range(8):
                k = g * 8 + j
                st, sp = k == 0, k == 31
                nc.tensor.matmul(xq_ps, xT[:, k], wq_t[:, j], start=st, stop=sp)
                nc.tensor.matmul(xk_ps, xT[:, k], wk_sb[:, k], start=st, stop=sp)
                nc.tensor.matmul(xv_ps, xT[:, k], wv_sb[:, k], start=st, stop=sp)

        # ---- RoPE (interleaved) on xq, xk; xv plain copy ----
        q_rope = const.tile([64, HQ * D], f32)
        k_rope = const.tile([64, D], f32)
        xv_sb = const.tile([64, D], f32)
        nc.vector.tensor_copy(out=xv_sb, in_=xv_ps)

        cosb4 = _sub_ap(cos_sb[:], [[0, HQ], [1, 64]])
        sinb4 = _sub_ap(sin_sb[:], [[0, HQ], [1, 64]])
        q_te = _sub_ap(xq_ps[:], [[128, HQ], [2, 64]])
        q_to = _sub_ap(xq_ps[:], [[128, HQ], [2, 64]], extra_offset=1)
        qr_te = _sub_ap(q_rope[:], [[128, HQ], [2, 64]])
        qr_to = _sub_ap(q_rope[:], [[128, HQ], [2, 64]], extra_offset=1)
        t1 = ropep.tile([64, HQ, 64], f32)
        t2 = ropep.tile([64, HQ, 64], f32)
        t3 = ropep.tile([64, HQ, 64], f32)
        t4 = ropep.tile([64, HQ, 64], f32)
        nc.vector.tensor_mul(t1, q_te, cosb4)
        nc.vector.tensor_mul(t2, q_to, sinb4)
        nc.vector.tensor_sub(qr_te, t1[:], t2[:])
        nc.vector.tensor_mul(t3, q_to, cosb4)
        nc.vector.tensor_mul(t4, q_te, sinb4)
        nc.vector.tensor_add(qr_to, t3[:], t4[:])

        cosb1 = _sub_ap(cos_sb[:], [[1, 64]])
        sinb1 = _sub_ap(sin_sb[:], [[1, 64]])
        k_te = _sub_ap(xk_ps[:], [[2, 64]])
        k_to = _sub_ap(xk_ps[:], [[2, 64]], extra_offset=1)
        kr_te = _sub_ap(k_rope[:], [[2, 64]])
        kr_to = _sub_ap(k_rope[:], [[2, 64]], extra_offset=1)
        s1 = ropep.tile([64, 64], f32)
        s2 = ropep.tile([64, 64], f32)
        s3 = ropep.tile([64, 64], f32)
        s4 = ropep.tile([64, 64], f32)
        nc.vector.tensor_mul(s1, k_te, cosb1)
        nc.vector.tensor_mul(s2, k_to, sinb1)
        nc.vector.tensor_sub(kr_te, s1[:], s2[:])
        nc.vector.tensor_mul(s3, k_to, cosb1)
        nc.vector.tensor_mul(s4, k_te, sinb1)
        nc.vector.tensor_add(kr_to, s3[:], s4[:])

        # ---- transpose q, k_new to [d, cols] layouts ----
        # qT: [128 d, b*16 + h*4 + i] so the per-b moving operand is one
        # contiguous [128, 16] slice (matmul RHS must be single-free-dim)
        qT = const.tile([128, COLS], f32)
        for h in range(HQ):
            pt = ptr.tile([128, 64], f32, tag="ptq")
            nc.tensor.transpose(pt, q_rope[:, h * 128 : (h + 1) * 128], ident)
            qT_dst = _sub_ap(qT[:], [[16, B], [1, QL]], extra_offset=h * QL)
            nc.vector.tensor_copy(
                out=qT_dst, in_=pt[:].rearrange("p (b i) -> p b i", i=QL)
            )
        kTn = const.tile([128, 64], f32)
        pt = ptr.tile([128, 64], f32, tag="ptq")
        nc.tensor.transpose(pt, k_rope, ident)
        nc.vector.tensor_copy(out=kTn, in_=pt)

        # xv rows regrouped so each b's 4 rows start at partition 0:
        # xv_rows[i, b, d] = xv[b*4+i, d]. SBUF APs cannot regroup the
        # partition dim, so bounce through DRAM (free-form APs there).
        xv_scratch = nc.dram_tensor("xv_scratch", [B * QL, D], f32).ap()
        nc.sync.dma_start(out=xv_scratch, in_=xv_sb[:])
        xv_rows = const.tile([QL, B, D], f32)
        nc.sync.dma_start(
            out=xv_rows, in_=xv_scratch.rearrange("(b i) d -> i b d", i=QL)
        )

        def qT_b(b):
            return qT[:, b * 16 : (b + 1) * 16]

        # phase A PSUM done (x^T, projections, small transposes)
        psA.close()
        psB = ctx.enter_context(ExitStack())
        psc = psB.enter_context(tc.tile_pool(name="psc", bufs=2, space="PSUM"))
        pacc = psB.enter_context(tc.tile_pool(name="pacc", bufs=1, space="PSUM"))

        # ---- new-key scores (all b): causal 4x4 per (b,h) ----
        ps_new = pacc.tile([QL, COLS], f32)
        for b in range(B):
            nc.tensor.matmul(
                ps_new[:, b * 16 : (b + 1) * 16],
                kTn[:, b * QL : (b + 1) * QL],
                qT_b(b),
                start=True,
                stop=True,
            )
        probs_new = const.tile([QL, COLS], f32)
        nc.scalar.activation(out=probs_new, in_=ps_new, func=Exp, scale=SCALE)
        nc.vector.tensor_mul(probs_new, probs_new[:], nmask_sb[:])

        # ---- ragged attention over the old cache, pipelined per sequence ----
        pv_ps = pacc.tile([128, COLS], f32)
        sums_ps = pacc.tile([1, COLS], f32)

        kT_tiles = {}
        v_tiles = {}
        probs_tiles = {}
        sc_tiles = {}

        def emit_load_scores(b):
            L0, nJ = L0s[b], nJs[b]
            if nJ == 0:
                return
            kT_t = kvp.tile([128, max_nJ * 128], f32, tag="kT")
            nc.sync.dma_start(out=kT_t[:, :L0], in_=kT_d[b, :, :L0])
            v_t = kvp.tile([128, max_nJ, D], f32, tag="v")
            nfull, tail = L0 // 128, L0 % 128
            if nfull:
                nc.sync.dma_start(
                    out=v_t[:, :nfull, :],
                    in_=v_d[b, : nfull * 128, :].rearrange("(s p) d -> p s d", p=128),
                )
            if tail:
                nc.sync.dma_start(
                    out=v_t[:tail, nfull, :], in_=v_d[b, nfull * 128 : L0, :]
                )
            sc = psc.tile([128, max_nJ * 16], f32, tag="sc")
            qb = qT_b(b)
            if tail:
                # pre-fill the tail chunk's columns with -1e30 so exp() zeroes
                # the unused partitions; the matmul overwrites rows [0, tail).
                nc.vector.memset(sc[:, (nJ - 1) * 16 : nJ * 16], -1e30)
            for s in range(nJ):
                cj = min(128, L0 - s * 128)
                nc.tensor.matmul(
                    sc[0:cj, s * 16 : (s + 1) * 16],
                    kT_t[:, s * 128 : s * 128 + cj],
                    qb,
                    start=True,
                    stop=True,
                )
            probs = probsp.tile([128, max_nJ * 16], f32, tag="probs")
            nc.scalar.activation(
                out=probs[:, : nJ * 16], in_=sc[:, : nJ * 16], func=Exp, scale=SCALE
            )
            kT_tiles[b], v_tiles[b], probs_tiles[b], sc_tiles[b] = kT_t, v_t, probs, sc

        def emit_sums_pv(b):
            L0, nJ = L0s[b], nJs[b]
            c0, c1 = b * 16, (b + 1) * 16
            probs = probs_tiles.get(b)
            v_t = v_tiles.get(b)
            # sums of exp via ones-matmul (garbage rows were exp(-1e30)=0)
            for s in range(nJ):
                nc.tensor.matmul(
                    sums_ps[0:1, c0:c1],
                    ones128,
                    probs[:, s * 16 : (s + 1) * 16],
                    start=(s == 0),
                    stop=False,
                )
            nc.tensor.matmul(
                sums_ps[0:1, c0:c1],
                ones4,
                probs_new[:, c0:c1],
                start=(nJ == 0),
                stop=True,
            )
            # PV accumulation: out^T[d, (h,i)] += V^T-free chunks
            for s in range(nJ):
                cj = min(128, L0 - s * 128)
                nc.tensor.matmul(
                    pv_ps[:, c0:c1],
                    v_t[0:cj, s, :],
                    probs[0:cj, s * 16 : (s + 1) * 16],
                    start=(s == 0),
                    stop=False,
                )
            nc.tensor.matmul(
                pv_ps[:, c0:c1],
                xv_rows[:, b, :],
                probs_new[:, c0:c1],
                start=(nJ == 0),
                stop=True,
            )

        for b in range(B):
            emit_load_scores(b)
            if b > 0:
                emit_sums_pv(b - 1)
        emit_sums_pv(B - 1)

        # ---- finalize: attnT = pv / sums ----
        sums_sb = fin.tile([1, COLS], f32)
        nc.vector.tensor_copy(out=sums_sb, in_=sums_ps)
        recip = fin.tile([1, COLS], f32)
        nc.vector.reciprocal(out=recip, in_=sums_sb[:])
        bc_ps = pacc.tile([128, COLS], f32)
        nc.tensor.matmul(bc_ps, ones_row, recip[:], start=True, stop=True)
        bc_sb = fin.tile([128, COLS], f32)
        nc.vector.tensor_copy(out=bc_sb, in_=bc_ps)
        # attnT in h-major cols (h*64 + b*4 + i) so the wo matmul lhsT is a
        # contiguous [128, 64] slice; the divide does the (b,h) permute.
        attnT = fin.tile([128, COLS], f32)
        attnT_dst = _sub_ap(attnT[:], [[4, B], [64, HQ], [1, QL]])
        nc.vector.tensor_mul(
            attnT_dst,
            pv_ps[:].rearrange("p (b h i) -> p b h i", h=HQ, i=QL),
            bc_sb[:].rearrange("p (b h i) -> p b h i", h=HQ, i=QL),
        )

        def attnT_h(h):
            return attnT[:, h * 64 : (h + 1) * 64]

        # phase B PSUM done (attention)
        psB.close()
        py = ctx.enter_context(tc.tile_pool(name="py", bufs=1, space="PSUM"))

        # ---- output projection: y[64, 4096] = attn[64, 512] @ wo ----
        y_banks = [
            py.tile([64, 512], f32, tag=f"y{nt}", name=f"y_bank{nt}")
            for nt in range(8)
        ]
        for h in range(HQ):
            wo_t = wstream.tile([128, 8, 512], f32, tag="w")
            nc.sync.dma_start(
                out=wo_t,
                in_=wo_d[h * 128 : (h + 1) * 128, :].rearrange(
                    "p (n d) -> p n d", d=512
                ),
            )
            for nt in range(8):
                nc.tensor.matmul(
                    y_banks[nt],
                    attnT_h(h),
                    wo_t[:, nt, :],
                    start=(h == 0),
                    stop=(h == HQ - 1),
                )
        for nt in range(8):
            y_sb = yp.tile([64, 512], f32, tag="y_sb")
            nc.vector.tensor_copy(out=y_sb, in_=y_banks[nt])
            nc.sync.dma_start(out=out_d[:, nt * 512 : (nt + 1) * 512], in_=y_sb)

    nc.compile()
    return nc


_CACHE = {}


def _get_nc(cache_len):
    key = tuple(int(v) for v in cache_len)
    if key not in _CACHE:
        _CACHE[key] = _build(cache_len)
    return _CACHE[key]


def _prep_shards(x, wq, wk, wv, wo, cache_k, cache_v, cache_len):
    x = np.ascontiguousarray(x, dtype=np.float32)
    cache_len = np.asarray(cache_len, dtype=np.int32)

    pos = (cache_len[:, None].astype(np.int64) + np.arange(QL)[None, :]).reshape(-1)
    inv_freq = 1.0 / (THETA ** (np.arange(D // 2, dtype=np.float64) / (D // 2)))
    ang = pos[:, None] * inv_freq[None, :]
    cosb = np.cos(ang).astype(np.float32)
    sinb = np.sin(ang).astype(np.float32)

    nmask = np.zeros((QL, COLS), dtype=np.float32)
    for j in range(QL):
        for col in range(COLS):
            i = col % QL
            if j <= i:
                nmask[j, col] = 1.0

    kT_all = np.ascontiguousarray(
        np.transpose(cache_k, (2, 0, 3, 1)), dtype=np.float32
    )  # [KVH, B, D, KV]
    v_all = np.ascontiguousarray(
        np.transpose(cache_v, (2, 0, 1, 3)), dtype=np.float32
    )  # [KVH, B, KV, D]

    in_maps = []
    for c in range(N_CORES):
        in_maps.append(
            {
                "x": x,
                "wq": np.ascontiguousarray(wq[:, c * 512 : (c + 1) * 512], np.float32),
                "wk": np.ascontiguousarray(wk[:, c * 128 : (c + 1) * 128], np.float32),
                "wv": np.ascontiguousarray(wv[:, c * 128 : (c + 1) * 128], np.float32),
                "wo": np.ascontiguousarray(wo[c * 512 : (c + 1) * 512, :], np.float32),
                "kT": kT_all[c],
                "v": v_all[c],
                "cosb": cosb,
                "sinb": sinb,
                "nmask": nmask,
            }
        )
    return in_maps, cache_len


def _run(inputs, trace=False, trace_kwargs=None):
    _install_ntff_hook()
    from concourse.bass_utils import run_bass_kernel_spmd

    in_maps, cache_len = _prep_shards(**inputs)
    nc = _get_nc(cache_len)
    res = run_bass_kernel_spmd(
        nc,
        in_maps,
        core_ids=list(range(N_CORES)),
        trace=trace,
        **(trace_kwargs or {}),
    )
    out = np.zeros((B * QL, DIM), dtype=np.float32)
    for i in range(N_CORES):
        out += res.results[i]["out"]
    return out, res


def kernel(**inputs):
    out, _ = _run(inputs, trace=False)
    return out


def kernel_profiled(**inputs):
    out, res = _run(inputs, trace=True)
    return out, res


# revision 14
# speedup vs baseline: 2.7237x; 2.7237x over previous
"""Trainium2 Bass kernel for ragged GQA attention decode (B=16, QL=4, KV=4096,
H=32, KVH=8, D=128, DIM=4096), tensor-parallel over 8 NeuronCores.

Sharding: core c owns q-heads [4c, 4c+4) and kv-head c. wq/wk/wv are
column-split, wo row-split, KV cache split along the kv-head dim. Each core
computes a partial [64, 4096] output (its heads through its wo rows); the
host sums the 8 partials.

The Bass graph is specialized to the actual cache_len values (known on host
at build time), so only the live prefix of the KV cache is ever read.

Compute runs in bf16 (f32 PSUM accumulation): the weights and KV cache are
shipped to the device as bf16 shards, halving HBM traffic and making the
TensorEngine matmuls single-pass.
"""

import math
import sys
import types

import numpy as np

B, QL, KV, H, KVH, D, DIM = 16, 4, 4096, 32, 8, 128, 4096
N_CORES = 8
HQ = H // N_CORES  # 4 q heads per core
COLS = B * HQ * QL  # 256 = (b, h, i) columns of the per-core attention state
THETA = 10000.0
SCALE = 1.0 / math.sqrt(D)
NJMAX = KV // 128  # 32


def _install_ntff_hook():
    """Make run_bass_kernel_spmd(trace=True) work in this image: register the
    NTFF profile hook that trn_boot could not (antenv.axon_hooks missing)."""
    try:
        from antenv.axon_hooks import get_axon_ntff_profile_hook  # noqa: F401

        return
    except ImportError:
        pass
    try:
        import antenv
        from trn_agent_boot.trn_boot import _ntff_profile_via_ctypes

        hook = _ntff_profile_via_ctypes("/opt/axon/libaxon_pjrt.so")
        mod = types.ModuleType("antenv.axon_hooks")
        mod.get_axon_ntff_profile_hook = lambda: hook
        mod.set_axon_ntff_profile_hook = lambda h: None
        sys.modules["antenv.axon_hooks"] = mod
        antenv.axon_hooks = mod
    except Exception:
        pass


def _sub_ap(ap, free_dims, extra_offset=0):
    """AP with the same tensor/partition dim but custom free [step, count] dims."""
    import concourse.bass as bass

    return bass.AP(
        tensor=ap.tensor, offset=ap.offset + extra_offset, ap=[ap.ap[0]] + free_dims
    )


def _build(cache_len):
    """Build the per-core Bacc graph, specialized to cache_len (np.int array [B])."""
    import concourse.bacc as bacc
    import concourse.mybir as mybir
    import concourse.tile as tile
    from concourse.masks import make_identity
    from contextlib import ExitStack

    f32 = mybir.dt.float32
    bf16 = mybir.dt.bfloat16
    Exp = mybir.ActivationFunctionType.Exp

    nc = bacc.Bacc("TRN2", target_bir_lowering=False, debug=False, num_devices=N_CORES)

    x_d = nc.dram_tensor("x", [B * QL, DIM], f32, kind="ExternalInput").ap()
    wq_d = nc.dram_tensor("wq", [DIM, HQ * D], bf16, kind="ExternalInput").ap()
    wk_d = nc.dram_tensor("wk", [128, 32, D], bf16, kind="ExternalInput").ap()
    wv_d = nc.dram_tensor("wv", [128, 32, D], bf16, kind="ExternalInput").ap()
    wo_d = nc.dram_tensor("wo", [HQ * D, DIM], bf16, kind="ExternalInput").ap()
    kT_d = nc.dram_tensor("kT", [B, D, KV], bf16, kind="ExternalInput").ap()
    v_d = nc.dram_tensor("v", [B, 128, NJMAX, D], bf16, kind="ExternalInput").ap()
    cos_d = nc.dram_tensor("cosb", [B * QL, D // 2], f32, kind="ExternalInput").ap()
    sin_d = nc.dram_tensor("sinb", [B * QL, D // 2], f32, kind="ExternalInput").ap()
    nmask_d = nc.dram_tensor("nmask", [QL, COLS], bf16, kind="ExternalInput").ap()
    out_d = nc.dram_tensor("out", [B * QL, DIM], f32, kind="ExternalOutput").ap()

    L0s = [int(v) for v in cache_len]
    nJs = [(L + 127) // 128 for L in L0s]
    max_nJ = max(nJs) if nJs else 1

    with tile.TileContext(nc) as tc, ExitStack() as ctx:
        const = ctx.enter_context(tc.tile_pool(name="const", bufs=1))
        wstream = ctx.enter_context(tc.tile_pool(name="wstream", bufs=2))
        ropep = ctx.enter_context(tc.tile_pool(name="ropep", bufs=1))
        kvp = ctx.enter_context(tc.tile_pool(name="kvp", bufs=2))
        probsp = ctx.enter_context(tc.tile_pool(name="probsp", bufs=2))
        fin = ctx.enter_context(tc.tile_pool(name="fin", bufs=1))
        yp = ctx.enter_context(tc.tile_pool(name="yp", bufs=2))
        # PSUM pools are phase-scoped (stack allocator, 8 banks total)
        psA = ctx.enter_context(ExitStack())
        ptr = psA.enter_context(tc.tile_pool(name="ptr", bufs=2, space="PSUM"))
        pproj = psA.enter_context(tc.tile_pool(name="pproj", bufs=1, space="PSUM"))

        # ---- constants ----
        ident = const.tile([64, 64], f32)
        make_identity(nc, ident)
        ones128 = const.tile([128, 1], bf16)
        nc.vector.memset(ones128, 1.0)
        ones4 = const.tile([4, 1], bf16)
        nc.vector.memset(ones4, 1.0)
        ones_row = const.tile([1, 128], f32)
        nc.vector.memset(ones_row, 1.0)
        cos_sb = const.tile([64, 64], f32)
        nc.sync.dma_start(out=cos_sb, in_=cos_d)
        sin_sb = const.tile([64, 64], f32)
        nc.sync.dma_start(out=sin_sb, in_=sin_d)
        nmask_sb = const.tile([QL, COLS], bf16)
        nc.sync.dma_start(out=nmask_sb, in_=nmask_d)
        x_sb = const.tile([64, DIM], f32)
        nc.sync.dma_start(out=x_sb, in_=x_d)
        # prewarm the ACT exp table
        warm = const.tile([1, 1], f32)
        nc.scalar.activation(out=warm, in_=ones_row[0:1, 0:1], func=Exp)

        # ---- x^T: 32 PE transposes of [64,128] -> xT [128, 32, 64] (bf16) ----
        xT = const.tile([128, 32, 64], bf16)
        for g in range(4):
            pt = ptr.tile([128, 8, 64], f32, tag="ptx")
            for j in range(8):
                k = g * 8 + j
                nc.tensor.transpose(pt[:, j], x_sb[:, k * 128 : (k + 1) * 128], ident)
            nc.vector.tensor_copy(out=xT[:, g * 8 : (g + 1) * 8], in_=pt)

        # ---- QKV projections (orientation: out natural [64, cols]) ----
        wk_sb = const.tile([128, 32, D], bf16)
        nc.sync.dma_start(out=wk_sb, in_=wk_d)
        wv_sb = const.tile([128, 32, D], bf16)
        nc.sync.dma_start(out=wv_sb, in_=wv_d)

        xq_ps = pproj.tile([64, HQ * D], f32)
        xk_ps = pproj.tile([64, D], f32)
        xv_ps = pproj.tile([64, D], f32)
        for g in range(4):
            wq_t = wstream.tile([128, 8, HQ * D], bf16, tag="w")
            nc.sync.dma_start(
                out=wq_t,
                in_=wq_d[g * 1024 : (g + 1) * 1024, :].rearrange(
                    "(n p) d -> p n d", p=128
                ),
            )
            for j in range(8):
                k = g * 8 + j
                st, sp = k == 0, k == 31
                nc.tensor.matmul(xq_ps, xT[:, k], wq_t[:, j], start=st, stop=sp)
                nc.tensor.matmul(xk_ps, xT[:, k], wk_sb[:, k], start=st, stop=sp)
                nc.tensor.matmul(xv_ps, xT[:, k], wv_sb[:, k], start=st, stop=sp)

        # ---- RoPE (interleaved) on xq, xk; xv plain copy (cast bf16) ----
        q_rope = const.tile([64, HQ * D], f32)
        k_rope = const.tile([64, D], f32)
        xv_sb = const.tile([64, D], bf16)
        nc.vector.tensor_copy(out=xv_sb, in_=xv_ps)

        cosb4 = _sub_ap(cos_sb[:], [[0, HQ], [1, 64]])
        sinb4 = _sub_ap(sin_sb[:], [[0, HQ], [1, 64]])
        q_te = _sub_ap(xq_ps[:], [[128, HQ], [2, 64]])
        q_to = _sub_ap(xq_ps[:], [[128, HQ], [2, 64]], extra_offset=1)
        qr_te = _sub_ap(q_rope[:], [[128, HQ], [2, 64]])
        qr_to = _sub_ap(q_rope[:], [[128, HQ], [2, 64]], extra_offset=1)
        t1 = ropep.tile([64, HQ, 64], f32)
        t2 = ropep.tile([64, HQ, 64], f32)
        t3 = ropep.tile([64, HQ, 64], f32)
        t4 = ropep.tile([64, HQ, 64], f32)
        nc.vector.tensor_mul(t1, q_te, cosb4)
        nc.vector.tensor_mul(t2, q_to, sinb4)
        nc.vector.tensor_sub(qr_te, t1[:], t2[:])
        nc.vector.tensor_mul(t3, q_to, cosb4)
        nc.vector.tensor_mul(t4, q_te, sinb4)
        nc.vector.tensor_add(qr_to, t3[:], t4[:])

        cosb1 = _sub_ap(cos_sb[:], [[1, 64]])
        sinb1 = _sub_ap(sin_sb[:], [[1, 64]])
        k_te = _sub_ap(xk_ps[:], [[2, 64]])
        k_to = _sub_ap(xk_ps[:], [[2, 64]], extra_offset=1)
        kr_te = _sub_ap(k_rope[:], [[2, 64]])
        kr_to = _sub_ap(k_rope[:], [[2, 64]], extra_offset=1)
        s1 = ropep.tile([64, 64], f32)
        s2 = ropep.tile([64, 64], f32)
        s3 = ropep.tile([64, 64], f32)
        s4 = ropep.tile([64, 64], f32)
        nc.vector.tensor_mul(s1, k_te, cosb1)
        nc.vector.tensor_mul(s2, k_to, sinb1)
        nc.vector.tensor_sub(kr_te, s1[:], s2[:])
        nc.vector.tensor_mul(s3, k_to, cosb1)
        nc.vector.tensor_mul(s4, k_te, sinb1)
        nc.vector.tensor_add(kr_to, s3[:], s4[:])

        # ---- transpose q, k_new to [d, cols] layouts (cast bf16) ----
        # qT: [128 d, b*16 + h*4 + i] so the per-b moving operand is one
        # contiguous [128, 16] slice (matmul RHS must be single-free-dim)
        qT = const.tile([128, COLS], bf16)
        for h in range(HQ):
            pt = ptr.tile([128, 64], f32, tag="ptq")
            nc.tensor.transpose(pt, q_rope[:, h * 128 : (h + 1) * 128], ident)
            qT_dst = _sub_ap(qT[:], [[16, B], [1, QL]], extra_offset=h * QL)
            nc.vector.tensor_copy(
                out=qT_dst, in_=pt[:].rearrange("p (b i) -> p b i", i=QL)
            )
        kTn = const.tile([128, 64], bf16)
        pt = ptr.tile([128, 64], f32, tag="ptq")
        nc.tensor.transpose(pt, k_rope, ident)
        nc.vector.tensor_copy(out=kTn, in_=pt)

        # xv rows regrouped so each b's 4 rows start at partition 0:
        # xv_rows[i, b, d] = xv[b*4+i, d]. SBUF APs cannot regroup the
        # partition dim, so bounce through DRAM (free-form APs there).
        xv_scratch = nc.dram_tensor("xv_scratch", [B * QL, D], bf16).ap()
        nc.sync.dma_start(out=xv_scratch, in_=xv_sb[:])
        xv_rows = const.tile([QL, B, D], bf16)
        nc.sync.dma_start(
            out=xv_rows, in_=xv_scratch.rearrange("(b i) d -> i b d", i=QL)
        )

        def qT_b(b):
            return qT[:, b * 16 : (b + 1) * 16]

        # phase A PSUM done (x^T, projections, small transposes)
        psA.close()
        psB = ctx.enter_context(ExitStack())
        psc = psB.enter_context(tc.tile_pool(name="psc", bufs=2, space="PSUM"))
        pacc = psB.enter_context(tc.tile_pool(name="pacc", bufs=1, space="PSUM"))

        # ---- new-key scores (all b): causal 4x4 per (b,h) ----
        ps_new = pacc.tile([QL, COLS], f32)
        for b in range(B):
            nc.tensor.matmul(
                ps_new[:, b * 16 : (b + 1) * 16],
                kTn[:, b * QL : (b + 1) * QL],
                qT_b(b),
                start=True,
                stop=True,
            )
        probs_new = const.tile([QL, COLS], bf16)
        nc.scalar.activation(out=probs_new, in_=ps_new, func=Exp, scale=SCALE)
        nc.vector.tensor_mul(probs_new, probs_new[:], nmask_sb[:])

        # ---- ragged attention over the old cache, pipelined per sequence ----
        pv_ps = pacc.tile([128, COLS], f32)
        sums_ps = pacc.tile([1, COLS], f32)

        kT_tiles = {}
        v_tiles = {}
        probs_tiles = {}

        def emit_load_scores(b):
            L0, nJ = L0s[b], nJs[b]
            if nJ == 0:
                return
            kT_t = kvp.tile([128, max_nJ * 128], bf16, tag="kT")
            nc.sync.dma_start(out=kT_t[:, :L0], in_=kT_d[b, :, :L0])
            v_t = kvp.tile([128, max_nJ, D], bf16, tag="v")
            nc.sync.dma_start(out=v_t[:, :nJ, :], in_=v_d[b, :, :nJ, :])
            sc = psc.tile([128, max_nJ * 16], f32, tag="sc")
            qb = qT_b(b)
            tail = L0 % 128
            if tail:
                # pre-fill the tail chunk's columns with -1e30 so exp() zeroes
                # the unused partitions; the matmul overwrites rows [0, tail).
                nc.vector.memset(sc[:, (nJ - 1) * 16 : nJ * 16], -1e30)
            for s in range(nJ):
                cj = min(128, L0 - s * 128)
                nc.tensor.matmul(
                    sc[0:cj, s * 16 : (s + 1) * 16],
                    kT_t[:, s * 128 : s * 128 + cj],
                    qb,
                    start=True,
                    stop=True,
                )
            probs = probsp.tile([128, max_nJ * 16], bf16, tag="probs")
            nc.scalar.activation(
                out=probs[:, : nJ * 16], in_=sc[:, : nJ * 16], func=Exp, scale=SCALE
            )
            kT_tiles[b], v_tiles[b], probs_tiles[b] = kT_t, v_t, probs

        def emit_sums_pv(b):
            L0, nJ = L0s[b], nJs[b]
            c0, c1 = b * 16, (b + 1) * 16
            probs = probs_tiles.get(b)
            v_t = v_tiles.get(b)
            # sums of exp via ones-matmul (garbage rows were exp(-1e30)=0)
            for s in range(nJ):
                nc.tensor.matmul(
                    sums_ps[0:1, c0:c1],
                    ones128,
                    probs[:, s * 16 : (s + 1) * 16],
                    start=(s == 0),
                    stop=False,
                )
            nc.tensor.matmul(
                sums_ps[0:1, c0:c1],
                ones4,
                probs_new[:, c0:c1],
                start=(nJ == 0),
                stop=True,
            )
            # PV accumulation: out^T[d, (h,i)] += V chunks^T . probs chunks
            for s in range(nJ):
                cj = min(128, L0 - s * 128)
                nc.tensor.matmul(
                    pv_ps[:, c0:c1],
                    v_t[0:cj, s, :],
                    probs[0:cj, s * 16 : (s + 1) * 16],
                    start=(s == 0),
                    stop=False,
                )
            nc.tensor.matmul(
                pv_ps[:, c0:c1],
                xv_rows[:, b, :],
                probs_new[:, c0:c1],
                start=(nJ == 0),
                stop=True,
            )

        for b in range(B):
            emit_load_scores(b)
            if b > 0:
                emit_sums_pv(b - 1)
        emit_sums_pv(B - 1)

        # ---- finalize: attnT = pv / sums ----
        sums_sb = fin.tile([1, COLS], f32)
        nc.vector.tensor_copy(out=sums_sb, in_=sums_ps)
        recip = fin.tile([1, COLS], f32)
        nc.vector.reciprocal(out=recip, in_=sums_sb[:])
        bc_ps = pacc.tile([128, COLS], f32)
        nc.tensor.matmul(bc_ps, ones_row, recip[:], start=True, stop=True)
        bc_sb = fin.tile([128, COLS], f32)
        nc.vector.tensor_copy(out=bc_sb, in_=bc_ps)
        # attnT in h-major cols (h*64 + b*4 + i) so the wo matmul lhsT is a
        # contiguous [128, 64] slice; the divide does the (b,h) permute.
        attnT = fin.tile([128, COLS], bf16)
        attnT_dst = _sub_ap(attnT[:], [[4, B], [64, HQ], [1, QL]])
        nc.vector.tensor_mul(
            attnT_dst,
            pv_ps[:].rearrange("p (b h i) -> p b h i", h=HQ, i=QL),
            bc_sb[:].rearrange("p (b h i) -> p b h i", h=HQ, i=QL),
        )

        def attnT_h(h):
            return attnT[:, h * 64 : (h + 1) * 64]

        # phase B PSUM done (attention)
        psB.close()
        py = ctx.enter_context(tc.tile_pool(name="py", bufs=1, space="PSUM"))

        # ---- output projection: y[64, 4096] = attn[64, 512] @ wo ----
        y_banks = [
            py.tile([64, 512], f32, tag=f"y{nt}", name=f"y_bank{nt}")
            for nt in range(8)
        ]
        for h in range(HQ):
            wo_t = wstream.tile([128, 8, 512], bf16, tag="w")
            nc.sync.dma_start(
                out=wo_t,
                in_=wo_d[h * 128 : (h + 1) * 128, :].rearrange(
                    "p (n d) -> p n d", d=512
                ),
            )
            for nt in range(8):
                nc.tensor.matmul(
                    y_banks[nt],
                    attnT_h(h),
                    wo_t[:, nt, :],
                    start=(h == 0),
                    stop=(h == HQ - 1),
                )
        for nt in range(8):
            y_sb = yp.tile([64, 512], f32, tag="y_sb")
            nc.vector.tensor_copy(out=y_sb, in_=y_banks[nt])
            nc.sync.dma_start(out=out_d[:, nt * 512 : (nt + 1) * 512], in_=y_sb)

    nc.compile()
    return nc


_CACHE = {}


def _get_nc(cache_len):
    key = tuple(int(v) for v in cache_len)
    if key not in _CACHE:
        _CACHE[key] = _build(cache_len)
    return _CACHE[key]


def _prep_shards(x, wq, wk, wv, wo, cache_k, cache_v, cache_len):
    import concourse.mybir as mybir

    bf16 = mybir.dt.np(mybir.dt.bfloat16)

    x = np.ascontiguousarray(x, dtype=np.float32)
    cache_len = np.asarray(cache_len, dtype=np.int32)

    pos = (cache_len[:, None].astype(np.int64) + np.arange(QL)[None, :]).reshape(-1)
    inv_freq = 1.0 / (THETA ** (np.arange(D // 2, dtype=np.float64) / (D // 2)))
    ang = pos[:, None] * inv_freq[None, :]
    cosb = np.cos(ang).astype(np.float32)
    sinb = np.sin(ang).astype(np.float32)

    nmask = np.zeros((QL, COLS), dtype=np.float32)
    for j in range(QL):
        for col in range(COLS):
            if j <= col % QL:
                nmask[j, col] = 1.0
    nmask = nmask.astype(bf16)

    # K^T per kv-head: [KVH, B, D, KV]; V swizzled so bf16 DMA runs stay long:
    # v_all[c, b, p, s, d] = V[c, b, s*128+p, d]
    kT_all = np.ascontiguousarray(np.transpose(cache_k, (2, 0, 3, 1))).astype(bf16)
    v_all = np.ascontiguousarray(
        np.transpose(
            cache_v.reshape(B, NJMAX, 128, KVH, D), (3, 0, 2, 1, 4)
        )
    ).astype(bf16)  # [KVH, B, 128, NJMAX, D]

    in_maps = []
    for c in range(N_CORES):
        wk_c = wk[:, c * 128 : (c + 1) * 128].reshape(32, 128, 128)
        wv_c = wv[:, c * 128 : (c + 1) * 128].reshape(32, 128, 128)
        in_maps.append(
            {
                "x": x,
                "wq": np.ascontiguousarray(wq[:, c * 512 : (c + 1) * 512]).astype(
                    bf16
                ),
                "wk": np.ascontiguousarray(np.transpose(wk_c, (1, 0, 2))).astype(bf16),
                "wv": np.ascontiguousarray(np.transpose(wv_c, (1, 0, 2))).astype(bf16),
                "wo": np.ascontiguousarray(wo[c * 512 : (c + 1) * 512, :]).astype(
                    bf16
                ),
                "kT": kT_all[c],
                "v": v_all[c],
                "cosb": cosb,
                "sinb": sinb,
                "nmask": nmask,
            }
        )
    return in_maps, cache_len


def _run(inputs, trace=False, trace_kwargs=None):
    _install_ntff_hook()
    from concourse.bass_utils import run_bass_kernel_spmd

    in_maps, cache_len = _prep_shards(**inputs)
    nc = _get_nc(cache_len)
    res = run_bass_kernel_spmd(
        nc,
        in_maps,
        core_ids=list(range(N_CORES)),
        trace=trace,
        **(trace_kwargs or {}),
    )
    out = np.zeros((B * QL, DIM), dtype=np.float32)
    for i in range(N_CORES):
        out += res.results[i]["out"]
    return out, res


def kernel(**inputs):
    out, _ = _run(inputs, trace=False)
    return out


def kernel_profiled(**inputs):
    out, res = _run(inputs, trace=True)
    return out, res


# revision 15
# speedup vs baseline: 3.2270x; 1.1848x over previous
"""Trainium2 Bass kernel for ragged GQA attention decode (B=16, QL=4, KV=4096,
H=32, KVH=8, D=128, DIM=4096), tensor-parallel over 8 NeuronCores.

Sharding: core c owns q-heads [4c, 4c+4) and kv-head c. wq/wk/wv are
column-split, wo row-split, KV cache split along the kv-head dim. Each core
computes a partial [64, 4096] output (its heads through its wo rows); the
host sums the 8 partials.

The Bass graph is specialized to the actual cache_len values (known on host
at build time), so only the live prefix of the KV cache is ever read.

Compute runs in bf16 (f32 PSUM accumulation): the weights and KV cache are
shipped to the device as bf16 shards, halving HBM traffic and making the
TensorEngine matmuls single-pass.
"""

import math
import sys
import types

import numpy as np

B, QL, KV, H, KVH, D, DIM = 16, 4, 4096, 32, 8, 128, 4096
N_CORES = 8
HQ = H // N_CORES  # 4 q heads per core
COLS = B * HQ * QL  # 256 = (b, h, i) columns of the per-core attention state
THETA = 10000.0
SCALE = 1.0 / math.sqrt(D)
NJMAX = KV // 128  # 32


def _install_ntff_hook():
    """Make run_bass_kernel_spmd(trace=True) work in this image: register the
    NTFF profile hook that trn_boot could not (antenv.axon_hooks missing)."""
    try:
        from antenv.axon_hooks import get_axon_ntff_profile_hook  # noqa: F401

        return
    except ImportError:
        pass
    try:
        import antenv
        from trn_agent_boot.trn_boot import _ntff_profile_via_ctypes

        hook = _ntff_profile_via_ctypes("/opt/axon/libaxon_pjrt.so")
        mod = types.ModuleType("antenv.axon_hooks")
        mod.get_axon_ntff_profile_hook = lambda: hook
        mod.set_axon_ntff_profile_hook = lambda h: None
        sys.modules["antenv.axon_hooks"] = mod
        antenv.axon_hooks = mod
    except Exception:
        pass


def _sub_ap(ap, free_dims, extra_offset=0):
    """AP with the same tensor/partition dim but custom free [step, count] dims."""
    import concourse.bass as bass

    return bass.AP(
        tensor=ap.tensor, offset=ap.offset + extra_offset, ap=[ap.ap[0]] + free_dims
    )


def _build(cache_len):
    """Build the per-core Bacc graph, specialized to cache_len (np.int array [B])."""
    import concourse.bacc as bacc
    import concourse.mybir as mybir
    import concourse.tile as tile
    from concourse.masks import make_identity
    from contextlib import ExitStack

    f32 = mybir.dt.float32
    bf16 = mybir.dt.bfloat16
    Exp = mybir.ActivationFunctionType.Exp

    nc = bacc.Bacc("TRN2", target_bir_lowering=False, debug=False, num_devices=N_CORES)

    x_d = nc.dram_tensor("x", [B * QL, DIM], f32, kind="ExternalInput").ap()
    wq_d = nc.dram_tensor("wq", [DIM, HQ * D], bf16, kind="ExternalInput").ap()
    wk_d = nc.dram_tensor("wk", [128, 32, D], bf16, kind="ExternalInput").ap()
    wv_d = nc.dram_tensor("wv", [128, 32, D], bf16, kind="ExternalInput").ap()
    wo_d = nc.dram_tensor("wo", [HQ * D, DIM], bf16, kind="ExternalInput").ap()
    kT_d = nc.dram_tensor("kT", [B, D, KV], bf16, kind="ExternalInput").ap()
    v_d = nc.dram_tensor("v", [B, 128, NJMAX, D], bf16, kind="ExternalInput").ap()
    cos_d = nc.dram_tensor("cosb", [B * QL, D // 2], f32, kind="ExternalInput").ap()
    sin_d = nc.dram_tensor("sinb", [B * QL, D // 2], f32, kind="ExternalInput").ap()
    nmask_d = nc.dram_tensor("nmask", [QL, COLS], bf16, kind="ExternalInput").ap()
    out_d = nc.dram_tensor("out", [B * QL, DIM], f32, kind="ExternalOutput").ap()

    L0s = [int(v) for v in cache_len]
    nJs = [(L + 127) // 128 for L in L0s]
    max_nJ = max(nJs) if nJs else 1

    with tile.TileContext(nc) as tc, ExitStack() as ctx:
        const = ctx.enter_context(tc.tile_pool(name="const", bufs=1))
        wstream = ctx.enter_context(tc.tile_pool(name="wstream", bufs=4))
        ropep = ctx.enter_context(tc.tile_pool(name="ropep", bufs=1))
        kvp = ctx.enter_context(tc.tile_pool(name="kvp", bufs=4))
        probsp = ctx.enter_context(tc.tile_pool(name="probsp", bufs=3))
        fin = ctx.enter_context(tc.tile_pool(name="fin", bufs=1))
        yp = ctx.enter_context(tc.tile_pool(name="yp", bufs=2))
        # PSUM pools are phase-scoped (stack allocator, 8 banks total)
        psA = ctx.enter_context(ExitStack())
        ptr = psA.enter_context(tc.tile_pool(name="ptr", bufs=2, space="PSUM"))
        pproj = psA.enter_context(tc.tile_pool(name="pproj", bufs=1, space="PSUM"))

        # ---- constants ----
        ident = const.tile([64, 64], f32)
        make_identity(nc, ident)
        ones128 = const.tile([128, 1], bf16)
        nc.vector.memset(ones128, 1.0)
        ones4 = const.tile([4, 1], bf16)
        nc.vector.memset(ones4, 1.0)
        ones_row = const.tile([1, 128], f32)
        nc.vector.memset(ones_row, 1.0)
        cos_sb = const.tile([64, 64], f32)
        nc.sync.dma_start(out=cos_sb, in_=cos_d)
        sin_sb = const.tile([64, 64], f32)
        nc.sync.dma_start(out=sin_sb, in_=sin_d)
        nmask_sb = const.tile([QL, COLS], bf16)
        nc.sync.dma_start(out=nmask_sb, in_=nmask_d)
        x_sb = const.tile([64, DIM], f32)
        nc.sync.dma_start(out=x_sb, in_=x_d)
        # prewarm the ACT exp table
        warm = const.tile([1, 1], f32)
        nc.scalar.activation(out=warm, in_=ones_row[0:1, 0:1], func=Exp)

        # ---- x^T: 32 PE transposes of [64,128] -> xT [128, 32, 64] (bf16) ----
        xT = const.tile([128, 32, 64], bf16)
        for g in range(4):
            pt = ptr.tile([128, 8, 64], f32, tag="ptx")
            for j in range(8):
                k = g * 8 + j
                nc.tensor.transpose(pt[:, j], x_sb[:, k * 128 : (k + 1) * 128], ident)
            nc.vector.tensor_copy(out=xT[:, g * 8 : (g + 1) * 8], in_=pt)

        # ---- QKV projections (orientation: out natural [64, cols]) ----
        wk_sb = const.tile([128, 32, D], bf16)
        nc.sync.dma_start(out=wk_sb, in_=wk_d)
        wv_sb = const.tile([128, 32, D], bf16)
        nc.sync.dma_start(out=wv_sb, in_=wv_d)

        xq_ps = pproj.tile([64, HQ * D], f32)
        xk_ps = pproj.tile([64, D], f32)
        xv_ps = pproj.tile([64, D], f32)
        for g in range(4):
            wq_t = wstream.tile([128, 8, HQ * D], bf16, tag="w")
            nc.sync.dma_start(
                out=wq_t,
                in_=wq_d[g * 1024 : (g + 1) * 1024, :].rearrange(
                    "(n p) d -> p n d", p=128
                ),
            )
            for j in range(8):
                k = g * 8 + j
                st, sp = k == 0, k == 31
                nc.tensor.matmul(xq_ps, xT[:, k], wq_t[:, j], start=st, stop=sp)
                nc.tensor.matmul(xk_ps, xT[:, k], wk_sb[:, k], start=st, stop=sp)
                nc.tensor.matmul(xv_ps, xT[:, k], wv_sb[:, k], start=st, stop=sp)

        # ---- RoPE (interleaved) on xq, xk; xv plain copy (cast bf16) ----
        q_rope = const.tile([64, HQ * D], f32)
        k_rope = const.tile([64, D], f32)
        xv_sb = const.tile([64, D], bf16)
        nc.vector.tensor_copy(out=xv_sb, in_=xv_ps)

        cosb4 = _sub_ap(cos_sb[:], [[0, HQ], [1, 64]])
        sinb4 = _sub_ap(sin_sb[:], [[0, HQ], [1, 64]])
        q_te = _sub_ap(xq_ps[:], [[128, HQ], [2, 64]])
        q_to = _sub_ap(xq_ps[:], [[128, HQ], [2, 64]], extra_offset=1)
        qr_te = _sub_ap(q_rope[:], [[128, HQ], [2, 64]])
        qr_to = _sub_ap(q_rope[:], [[128, HQ], [2, 64]], extra_offset=1)
        t1 = ropep.tile([64, HQ, 64], f32)
        t2 = ropep.tile([64, HQ, 64], f32)
        t3 = ropep.tile([64, HQ, 64], f32)
        t4 = ropep.tile([64, HQ, 64], f32)
        nc.vector.tensor_mul(t1, q_te, cosb4)
        nc.vector.tensor_mul(t2, q_to, sinb4)
        nc.vector.tensor_sub(qr_te, t1[:], t2[:])
        nc.vector.tensor_mul(t3, q_to, cosb4)
        nc.vector.tensor_mul(t4, q_te, sinb4)
        nc.vector.tensor_add(qr_to, t3[:], t4[:])

        cosb1 = _sub_ap(cos_sb[:], [[1, 64]])
        sinb1 = _sub_ap(sin_sb[:], [[1, 64]])
        k_te = _sub_ap(xk_ps[:], [[2, 64]])
        k_to = _sub_ap(xk_ps[:], [[2, 64]], extra_offset=1)
        kr_te = _sub_ap(k_rope[:], [[2, 64]])
        kr_to = _sub_ap(k_rope[:], [[2, 64]], extra_offset=1)
        s1 = ropep.tile([64, 64], f32)
        s2 = ropep.tile([64, 64], f32)
        s3 = ropep.tile([64, 64], f32)
        s4 = ropep.tile([64, 64], f32)
        nc.vector.tensor_mul(s1, k_te, cosb1)
        nc.vector.tensor_mul(s2, k_to, sinb1)
        nc.vector.tensor_sub(kr_te, s1[:], s2[:])
        nc.vector.tensor_mul(s3, k_to, cosb1)
        nc.vector.tensor_mul(s4, k_te, sinb1)
        nc.vector.tensor_add(kr_to, s3[:], s4[:])

        # ---- transpose q, k_new to [d, cols] layouts (cast bf16) ----
        # qT: [128 d, b*16 + h*4 + i] so the per-b moving operand is one
        # contiguous [128, 16] slice (matmul RHS must be single-free-dim)
        qT = const.tile([128, COLS], bf16)
        for h in range(HQ):
            pt = ptr.tile([128, 64], f32, tag="ptq")
            nc.tensor.transpose(pt, q_rope[:, h * 128 : (h + 1) * 128], ident)
            qT_dst = _sub_ap(qT[:], [[16, B], [1, QL]], extra_offset=h * QL)
            nc.vector.tensor_copy(
                out=qT_dst, in_=pt[:].rearrange("p (b i) -> p b i", i=QL)
            )
        kTn = const.tile([128, 64], bf16)
        pt = ptr.tile([128, 64], f32, tag="ptq")
        nc.tensor.transpose(pt, k_rope, ident)
        nc.vector.tensor_copy(out=kTn, in_=pt)

        # xv rows regrouped so each b's 4 rows start at partition 0:
        # xv_rows[i, b, d] = xv[b*4+i, d]. SBUF APs cannot regroup the
        # partition dim, so bounce through DRAM (free-form APs there).
        xv_scratch = nc.dram_tensor("xv_scratch", [B * QL, D], bf16).ap()
        nc.sync.dma_start(out=xv_scratch, in_=xv_sb[:])
        xv_rows = const.tile([QL, B, D], bf16)
        nc.sync.dma_start(
            out=xv_rows, in_=xv_scratch.rearrange("(b i) d -> i b d", i=QL)
        )

        def qT_b(b):
            return qT[:, b * 16 : (b + 1) * 16]

        # phase A PSUM done (x^T, projections, small transposes)
        psA.close()
        psB = ctx.enter_context(ExitStack())
        psc = psB.enter_context(tc.tile_pool(name="psc", bufs=3, space="PSUM"))
        pacc = psB.enter_context(tc.tile_pool(name="pacc", bufs=1, space="PSUM"))

        # ---- new-key scores (all b): causal 4x4 per (b,h) ----
        ps_new = pacc.tile([QL, COLS], f32)
        for b in range(B):
            nc.tensor.matmul(
                ps_new[:, b * 16 : (b + 1) * 16],
                kTn[:, b * QL : (b + 1) * QL],
                qT_b(b),
                start=True,
                stop=True,
            )
        probs_new = const.tile([QL, COLS], bf16)
        nc.scalar.activation(out=probs_new, in_=ps_new, func=Exp, scale=SCALE)
        nc.vector.tensor_mul(probs_new, probs_new[:], nmask_sb[:])

        # ---- ragged attention over the old cache, pipelined per sequence ----
        pv_ps = pacc.tile([128, COLS], f32)
        sums_ps = pacc.tile([1, COLS], f32)

        kT_tiles = {}
        v_tiles = {}
        probs_tiles = {}

        def emit_load_scores(b):
            L0, nJ = L0s[b], nJs[b]
            if nJ == 0:
                return
            kT_t = kvp.tile([128, max_nJ * 128], bf16, tag="kT")
            nc.sync.dma_start(out=kT_t[:, :L0], in_=kT_d[b, :, :L0])
            v_t = kvp.tile([128, max_nJ, D], bf16, tag="v")
            nc.sync.dma_start(out=v_t[:, :nJ, :], in_=v_d[b, :, :nJ, :])
            sc = psc.tile([128, max_nJ * 16], f32, tag="sc")
            qb = qT_b(b)
            tail = L0 % 128
            if tail:
                # pre-fill the tail chunk's columns with -1e30 so exp() zeroes
                # the unused partitions; the matmul overwrites rows [0, tail).
                nc.vector.memset(sc[:, (nJ - 1) * 16 : nJ * 16], -1e30)
            for s in range(nJ):
                cj = min(128, L0 - s * 128)
                nc.tensor.matmul(
                    sc[0:cj, s * 16 : (s + 1) * 16],
                    kT_t[:, s * 128 : s * 128 + cj],
                    qb,
                    start=True,
                    stop=True,
                )
            probs = probsp.tile([128, max_nJ * 16], bf16, tag="probs")
            nc.scalar.activation(
                out=probs[:, : nJ * 16], in_=sc[:, : nJ * 16], func=Exp, scale=SCALE
            )
            kT_tiles[b], v_tiles[b], probs_tiles[b] = kT_t, v_t, probs

        def emit_sums_pv(b):
            L0, nJ = L0s[b], nJs[b]
            c0, c1 = b * 16, (b + 1) * 16
            probs = probs_tiles.get(b)
            v_t = v_tiles.get(b)
            # sums of exp via ones-matmul (garbage rows were exp(-1e30)=0)
            for s in range(nJ):
                nc.tensor.matmul(
                    sums_ps[0:1, c0:c1],
                    ones128,
                    probs[:, s * 16 : (s + 1) * 16],
                    start=(s == 0),
                    stop=False,
                )
            nc.tensor.matmul(
                sums_ps[0:1, c0:c1],
                ones4,
                probs_new[:, c0:c1],
                start=(nJ == 0),
                stop=True,
            )
            # PV accumulation: out^T[d, (h,i)] += V chunks^T . probs chunks
            for s in range(nJ):
                cj = min(128, L0 - s * 128)
                nc.tensor.matmul(
                    pv_ps[:, c0:c1],
                    v_t[0:cj, s, :],
                    probs[0:cj, s * 16 : (s + 1) * 16],
                    start=(s == 0),
                    stop=False,
                )
            nc.tensor.matmul(
                pv_ps[:, c0:c1],
                xv_rows[:, b, :],
                probs_new[:, c0:c1],
                start=(nJ == 0),
                stop=True,
            )

        for b in range(B):
            emit_load_scores(b)
            if b > 0:
                emit_sums_pv(b - 1)
        emit_sums_pv(B - 1)

        # ---- finalize: attnT = pv / sums ----
        sums_sb = fin.tile([1, COLS], f32)
        nc.vector.tensor_copy(out=sums_sb, in_=sums_ps)
        recip = fin.tile([1, COLS], f32)
        nc.vector.reciprocal(out=recip, in_=sums_sb[:])
        bc_ps = pacc.tile([128, COLS], f32)
        nc.tensor.matmul(bc_ps, ones_row, recip[:], start=True, stop=True)
        bc_sb = fin.tile([128, COLS], f32)
        nc.vector.tensor_copy(out=bc_sb, in_=bc_ps)
        # attnT in h-major cols (h*64 + b*4 + i) so the wo matmul lhsT is a
        # contiguous [128, 64] slice; the divide does the (b,h) permute.
        attnT = fin.tile([128, COLS], bf16)
        attnT_dst = _sub_ap(attnT[:], [[4, B], [64, HQ], [1, QL]])
        nc.vector.tensor_mul(
            attnT_dst,
            pv_ps[:].rearrange("p (b h i) -> p b h i", h=HQ, i=QL),
            bc_sb[:].rearrange("p (b h i) -> p b h i", h=HQ, i=QL),
        )

        def attnT_h(h):
            return attnT[:, h * 64 : (h + 1) * 64]

        # phase B PSUM done (attention)
        psB.close()
        py = ctx.enter_context(tc.tile_pool(name="py", bufs=1, space="PSUM"))

        # ---- output projection: y[64, 4096] = attn[64, 512] @ wo ----
        y_banks = [
            py.tile([64, 512], f32, tag=f"y{nt}", name=f"y_bank{nt}")
            for nt in range(8)
        ]
        wo_tiles = []
        for h in range(HQ):
            wo_t = wstream.tile([128, 8, 512], bf16, tag="w", name=f"wo_t{h}")
            nc.sync.dma_start(
                out=wo_t,
                in_=wo_d[h * 128 : (h + 1) * 128, :].rearrange(
                    "p (n d) -> p n d", d=512
                ),
            )
            wo_tiles.append(wo_t)
        # nt-outer so each PSUM bank completes early and its writeback
        # overlaps the remaining matmuls
        for nt in range(8):
            for h in range(HQ):
                nc.tensor.matmul(
                    y_banks[nt],
                    attnT_h(h),
                    wo_tiles[h][:, nt, :],
                    start=(h == 0),
                    stop=(h == HQ - 1),
                )
            y_sb = yp.tile([64, 512], f32, tag="y_sb")
            nc.vector.tensor_copy(out=y_sb, in_=y_banks[nt])
            nc.sync.dma_start(out=out_d[:, nt * 512 : (nt + 1) * 512], in_=y_sb)

    nc.compile()
    return nc


_CACHE = {}


def _get_nc(cache_len):
    key = tuple(int(v) for v in cache_len)
    if key not in _CACHE:
        _CACHE[key] = _build(cache_len)
    return _CACHE[key]


def _prep_shards(x, wq, wk, wv, wo, cache_k, cache_v, cache_len):
    import concourse.mybir as mybir

    bf16 = mybir.dt.np(mybir.dt.bfloat16)

    x = np.ascontiguousarray(x, dtype=np.float32)
    cache_len = np.asarray(cache_len, dtype=np.int32)

    pos = (cache_len[:, None].astype(np.int64) + np.arange(QL)[None, :]).reshape(-1)
    inv_freq = 1.0 / (THETA ** (np.arange(D // 2, dtype=np.float64) / (D // 2)))
    ang = pos[:, None] * inv_freq[None, :]
    cosb = np.cos(ang).astype(np.float32)
    sinb = np.sin(ang).astype(np.float32)

    nmask = np.zeros((QL, COLS), dtype=np.float32)
    for j in range(QL):
        for col in range(COLS):
            if j <= col % QL:
                nmask[j, col] = 1.0
    nmask = nmask.astype(bf16)

    # K^T per kv-head: [KVH, B, D, KV]; V swizzled so bf16 DMA runs stay long:
    # v_all[c, b, p, s, d] = V[c, b, s*128+p, d]
    kT_all = np.ascontiguousarray(np.transpose(cache_k, (2, 0, 3, 1))).astype(bf16)
    v_all = np.ascontiguousarray(
        np.transpose(
            cache_v.reshape(B, NJMAX, 128, KVH, D), (3, 0, 2, 1, 4)
        )
    ).astype(bf16)  # [KVH, B, 128, NJMAX, D]

    in_maps = []
    for c in range(N_CORES):
        wk_c = wk[:, c * 128 : (c + 1) * 128].reshape(32, 128, 128)
        wv_c = wv[:, c * 128 : (c + 1) * 128].reshape(32, 128, 128)
        in_maps.append(
            {
                "x": x,
                "wq": np.ascontiguousarray(wq[:, c * 512 : (c + 1) * 512]).astype(
                    bf16
                ),
                "wk": np.ascontiguousarray(np.transpose(wk_c, (1, 0, 2))).astype(bf16),
                "wv": np.ascontiguousarray(np.transpose(wv_c, (1, 0, 2))).astype(bf16),
                "wo": np.ascontiguousarray(wo[c * 512 : (c + 1) * 512, :]).astype(
                    bf16
                ),
                "kT": kT_all[c],
                "v": v_all[c],
                "cosb": cosb,
                "sinb": sinb,
                "nmask": nmask,
            }
        )
    return in_maps, cache_len


def _run(inputs, trace=False, trace_kwargs=None):
    _install_ntff_hook()
    from concourse.bass_utils import run_bass_kernel_spmd

    in_maps, cache_len = _prep_shards(**inputs)
    nc = _get_nc(cache_len)
    res = run_bass_kernel_spmd(
        nc,
        in_maps,
        core_ids=list(range(N_CORES)),
        trace=trace,
        **(trace_kwargs or {}),
    )
    out = np.zeros((B * QL, DIM), dtype=np.float32)
    for i in range(N_CORES):
        out += res.results[i]["out"]
    return out, res


def kernel(**inputs):
    out, _ = _run(inputs, trace=False)
    return out


def kernel_profiled(**inputs):
    out, res = _run(inputs, trace=True)
    return out, res
